# Initial kernel scaffold
#
"""VQ codebook kernel for 8 Trainium2 NeuronCores.

Problem: z [32, 256, 32, 32] f32, codebook [2048, 256] f32 ->
  (z_q [32, 256, 32, 32] f32, idx [32768] i32, loss scalar f32)

Sharding: data-parallel over batch. Core c handles batches [4c, 4c+4) =
4096 tokens. Codebook replicated. Host combines the scalar loss.

Per-core algorithm (all engines, reference-fp32-faithful):
  d[t,k] = fl( fl(t1[t] + t2[k]) - 2*z.e )   computed ENTIRELY on the PE:
    - G terms via exact fp32 emulation with float32r pairs
      (z = zh + zl exact, e2n = -2*e = eh + el exact;
       G2acc = zh.eh + zh.el + zl.eh, dropped zl.el <= 2^-26)
    - final K=3 bias matmul [t1h; t1l; ones] x [ones; ones; t2h] adds
      fl(t1 + t2h) with a single PSUM-grid rounding  -> psum = d
  rm = running-min scan of d (DVE tensor_tensor_scan)  -> m = rm[:, -1]
  -argmin = sum(Sign(m - rm)) via ACT accum_out (first-index ties exact)
  z_q rows via gpsimd indirect gather from the codebook in DRAM
  loss: host sums the per-token d_min values (loss = 1.25 * sum / (N*D))
"""
import sys

for _p in ("/opt/trn_rl_repo", "/root/.axon_site/_ro/trn_rl_repo"):
    if _p not in sys.path:
        sys.path.append(_p)

import numpy as np
from contextlib import ExitStack

import concourse.bass as bass
import concourse.tile as tile
from concourse import mybir, bacc
from concourse.bass_utils import run_bass_kernel_spmd
from concourse.masks import make_identity

F32 = mybir.dt.float32
F32R = mybir.dt.float32r
I32 = mybir.dt.int32
AL = mybir.AluOpType
ACTF = mybir.ActivationFunctionType

B, D, HW = 32, 256, 1024          # batch, channels, H*W
K = 2048                          # codes
NCORES = 8
BPC = B // NCORES                 # 4 batches/core
NTOK = BPC * HW                   # 4096 tokens/core
NT = NTOK // 128                  # 32 token tiles
BETA = 0.25

_CACHE = {}


def _build():
    nc = bacc.Bacc(trn_type="TRN2", target_bir_lowering=False)
    z_in = nc.declare_dram_parameter("z_in", [BPC, D, HW], F32, isOutput=False)
    cb_in = nc.declare_dram_parameter("cb_in", [K, D], F32, isOutput=False)
    zq_out = nc.declare_dram_parameter("zq_out", [NTOK, D], F32, isOutput=True)
    idx_out = nc.declare_dram_parameter("idx_out", [NTOK, 1], I32, isOutput=True)
    msum_out = nc.declare_dram_parameter("msum_out", [128, NT], F32, isOutput=True)

    with tile.TileContext(nc) as tc, ExitStack() as ctx:
        const = ctx.enter_context(tc.tile_pool(name="const", bufs=1))

        # ------- persistent tiles
        zh = [const.tile([128, NTOK], F32R, tag=f"zh{c}") for c in range(2)]
        zl = [const.tile([128, NTOK], F32R, tag=f"zl{c}") for c in range(2)]
        eh = [const.tile([128, K], F32R, tag=f"eh{c}") for c in range(2)]
        el = [const.tile([128, K], F32R, tag=f"el{c}") for c in range(2)]
        bias2 = const.tile([3, NTOK], F32R, tag="bias2")    # t1h / t1l / ones
        biasmv = const.tile([3, K], F32R, tag="biasmv")     # ones / ones / t2h
        neginf = const.tile([128, 1], F32, tag="neginf")
        msum = const.tile([128, NT], F32, tag="msum")
        negk = const.tile([128, NT], F32, tag="negk")
        idxi = const.tile([128, NT], I32, tag="idxi")
        ident = const.tile([128, 128], F32, tag="ident")

        nc.vector.memset(neginf[:], -3.0e38)
        make_identity(nc, ident[:])

        # ------- prologue
        with tc.tile_pool(name="pscr", bufs=1) as pscr, \
             tc.tile_pool(name="ppsum", bufs=1, space="PSUM") as ppsum:

            # z slab loads, d-major: [128 d, BPC, HW]
            zf = []
            for c in range(2):
                t = pscr.tile([128, BPC, HW], F32, tag=f"zf{c}")
                nc.sync.dma_start(
                    t[:], z_in[:, c * 128:(c + 1) * 128, :].rearrange("b d x -> d b x"))
                zf.append(t)

            # codebook: 16 tiles of [128 codes, 256 d] side by side
            cball = pscr.tile([128, 16 * D], F32, tag="cball")
            for i in range(16):
                nc.sync.dma_start(cball[:, i * D:(i + 1) * D],
                                  cb_in[i * 128:(i + 1) * 128, :])

            onescol_f = pscr.tile([128, 1], F32, tag="onescol_f")
            nc.vector.memset(onescol_f[:], 1.0)
            onescol = pscr.tile([128, 1], F32R, tag="onescol")
            nc.vector.tensor_copy(onescol[:], onescol_f[:])
            onesrow_f = pscr.tile([1, NTOK], F32, tag="onesrow_f")
            nc.vector.memset(onesrow_f[:], 1.0)
            nc.scalar.activation(out=bias2[2:3, :].bitcast(F32), in_=onesrow_f[:],
                                 func=ACTF.Copy)
            nc.gpsimd.dma_start(biasmv[0:1, :], bias2[2:3, 0:K])
            nc.gpsimd.dma_start(biasmv[1:2, :], bias2[2:3, 0:K])

            # z hi/lo pairs + squares
            zsq = []
            for c in range(2):
                zfc = zf[c][:].rearrange("p b x -> p (b x)")
                s = pscr.tile([128, NTOK], F32R, tag=f"zsq{c}")
                nc.scalar.activation(out=s[:], in_=zfc, func=ACTF.Square)
                zsq.append(s)
                nc.vector.tensor_copy(zh[c][:], zfc)
                nc.vector.tensor_tensor(out=zl[c][:], in0=zfc,
                                        in1=zh[c][:].bitcast(F32), op=AL.subtract)

            # t1 columns: t1col[p, t] = sum_d z[t*128+p, d]^2
            p_t1 = ppsum.tile([128, NT], F32, tag="p_t1")
            for t in range(NT):
                sl = slice(t * 128, (t + 1) * 128)
                nc.tensor.matmul(p_t1[:, t:t + 1], lhsT=zsq[0][:, sl],
                                 rhs=onescol[:], start=True, stop=False)
                nc.tensor.matmul(p_t1[:, t:t + 1], lhsT=zsq[1][:, sl],
                                 rhs=onescol[:], start=False, stop=True)
            t1col = pscr.tile([128, NT], F32, tag="t1col")
            nc.vector.tensor_copy(t1col[:], p_t1[:])
            t1h = pscr.tile([128, NT], F32R, tag="t1h")
            t1l = pscr.tile([128, NT], F32R, tag="t1l")
            nc.vector.tensor_copy(t1h[:], t1col[:])
            nc.vector.tensor_tensor(out=t1l[:], in0=t1col[:],
                                    in1=t1h[:].bitcast(F32), op=AL.subtract)

            # t2 columns: t2col[p, i] = |e_(i*128+p)|^2
            sqjunk = pscr.tile([128, D], F32, tag="sqjunk")
            t2col = pscr.tile([128, 16], F32, tag="t2col")
            for i in range(16):
                nc.scalar.activation(out=sqjunk[:], in_=cball[:, i * D:(i + 1) * D],
                                     func=ACTF.Square, accum_out=t2col[:, i:i + 1])
            t2h = pscr.tile([128, 16], F32R, tag="t2h")
            nc.vector.tensor_copy(t2h[:], t2col[:])

            # transpose t1h/t1l/t2h columns into the bias row tiles
            for (src, n, dst) in ((t1h, NT, bias2[0:1, :]),
                                  (t1l, NT, bias2[1:2, :]),
                                  (t2h, 16, biasmv[2:3, :])):
                p_tr = ppsum.tile([NT, 128], F32, tag="p_tr")
                nc.tensor.transpose(out=p_tr[:n, :], in_=src[:, :n].bitcast(F32),
                                    identity=ident[:])
                bnc = pscr.tile([NT, 128], F32R, tag="bnc")
                nc.vector.tensor_copy(bnc[:n, :], p_tr[:n, :])
                nc.gpsimd.dma_start(dst, bnc[:n, :].rearrange("a b -> (a b)")[None, :])

            # e2n = -2 * cb^T, split hi/lo, per d-chunk
            for c in range(2):
                p_e = ppsum.tile([128, K], F32, tag="p_e")
                for i in range(16):
                    nc.tensor.transpose(
                        out=p_e[:, i * 128:(i + 1) * 128],
                        in_=cball[:, i * D + c * 128: i * D + (c + 1) * 128],
                        identity=ident[:])
                e2n = pscr.tile([128, K], F32, tag="e2n")
                nc.scalar.activation(out=e2n[:], in_=p_e[:], func=ACTF.Copy,
                                     scale=-2.0)
                nc.vector.tensor_copy(eh[c][:], e2n[:])
                nc.vector.tensor_tensor(out=el[c][:], in0=e2n[:],
                                        in1=eh[c][:].bitcast(F32), op=AL.subtract)

        # ------- main loop
        with tc.tile_pool(name="work", bufs=3) as work, \
             tc.tile_pool(name="mpsum", bufs=2, space="PSUM") as mpsum:
            for t in range(NT):
                sl = slice(t * 128, (t + 1) * 128)
                pd = mpsum.tile([128, K], F32, tag="pd")
                # stationary-major: 6 G terms then the bias term, 4 banks each
                terms = [(zh[0], eh[0]), (zh[0], el[0]), (zl[0], eh[0]),
                         (zh[1], eh[1]), (zh[1], el[1]), (zl[1], eh[1])]
                for ti, (zz, ee) in enumerate(terms):
                    for b in range(4):
                        bs = slice(b * 512, (b + 1) * 512)
                        nc.tensor.matmul(pd[:, bs], lhsT=zz[:, sl], rhs=ee[:, bs],
                                         start=(ti == 0), stop=False)
                for b in range(4):
                    bs = slice(b * 512, (b + 1) * 512)
                    nc.tensor.matmul(pd[:, bs], lhsT=bias2[:, sl],
                                     rhs=biasmv[:, bs], start=False, stop=True)

                rm = work.tile([128, K], F32, tag="rm")
                nc.vector.tensor_tensor_scan(
                    out=rm[:], data0=pd[:], data1=neginf[:].to_broadcast([128, K]),
                    initial=3.0e38, op0=AL.min, op1=AL.max)
                nc.vector.tensor_copy(msum[:, t:t + 1], rm[:, K - 1:K])

                sgn = work.tile([128, K], F32, tag="sgn")
                nc.scalar.activation(
                    out=sgn[:], in_=rm[:], func=ACTF.Sign,
                    bias=rm[:, K - 1:K], scale=-1.0,
                    accum_out=negk[:, t:t + 1])

                nc.vector.tensor_scalar(out=idxi[:, t:t + 1], in0=negk[:, t:t + 1],
                                        scalar1=-1.0, scalar2=None, op0=AL.mult)

                zqt = work.tile([128, D], F32, tag="zqt")
                nc.gpsimd.indirect_dma_start(
                    out=zqt[:], out_offset=None, in_=cb_in[:],
                    in_offset=bass.IndirectOffsetOnAxis(ap=idxi[:, t:t + 1], axis=0))
                nc.sync.dma_start(zq_out[sl, :], zqt[:])
                nc.sync.dma_start(idx_out[sl, :], idxi[:, t:t + 1])

        nc.sync.dma_start(msum_out[:], msum[:])

    nc.compile()
    return nc


def _get_nc():
    if "nc" not in _CACHE:
        _CACHE["nc"] = _build()
    return _CACHE["nc"]


def kernel(z: np.ndarray, codebook: np.ndarray):
    z = np.ascontiguousarray(z, dtype=np.float32)
    codebook = np.ascontiguousarray(codebook, dtype=np.float32)
    nc = _get_nc()

    in_maps = []
    for c in range(NCORES):
        zslab = np.ascontiguousarray(
            z[c * BPC:(c + 1) * BPC].reshape(BPC, D, HW))
        in_maps.append({"z_in": zslab, "cb_in": codebook})

    res = run_bass_kernel_spmd(nc, in_maps, core_ids=list(range(NCORES)))
    outs = res.results

    zq_flat = np.concatenate([o["zq_out"] for o in outs], axis=0)   # [32768, 256]
    idx = np.concatenate([o["idx_out"][:, 0] for o in outs], axis=0).astype(np.int32)

    total = np.float64(0.0)
    for o in outs:
        # msum[p, t] = d_min of token t*128+p on that core
        total += o["msum_out"].astype(np.float64).sum()
    e32 = np.float32(total / (B * HW * D))
    loss = np.float32(e32 + np.float32(np.float32(BETA) * e32))

    zq = zq_flat.reshape(B, 32, 32, D).transpose(0, 3, 1, 2)
    zq = np.ascontiguousarray(zq, dtype=np.float32)
    return zq, idx, loss


# revision 6
# speedup vs baseline: 1.0386x; 1.0386x over previous
"""VQ codebook kernel for 8 Trainium2 NeuronCores.

Problem: z [32, 256, 32, 32] f32, codebook [2048, 256] f32 ->
  (z_q [32, 256, 32, 32] f32, idx [32768] i32, loss scalar f32)

Sharding: data-parallel over batch. Core c handles batches [4c, 4c+4) =
4096 tokens. Codebook replicated. Host combines the scalar loss.

Per-core algorithm (all engines, reference-fp32-faithful):
  d[t,k] = fl( fl(t1[t] + t2[k]) - 2*z.e )   computed ENTIRELY on the PE:
    - G terms via exact fp32 emulation with float32r pairs
      (z = zh + zl exact, e2n = -2*e = eh + el exact;
       G2acc = zh.eh + zh.el + zl.eh, dropped zl.el <= 2^-26)
    - final K=3 bias matmul [t1h; t1l; ones] x [ones; ones; t2h] adds
      fl(t1 + t2h) with a single PSUM-grid rounding  -> psum = d
  rm = running-min scan of d (DVE tensor_tensor_scan)  -> m = rm[:, -1]
  -argmin = sum(Sign(m - rm)) via ACT accum_out (first-index ties exact)
  z_q rows via gpsimd indirect gather from the codebook in DRAM
  loss: host sums the per-token d_min values (loss = 1.25 * sum / (N*D))
"""
import sys

for _p in ("/opt/trn_rl_repo", "/root/.axon_site/_ro/trn_rl_repo"):
    if _p not in sys.path:
        sys.path.append(_p)

import numpy as np
from contextlib import ExitStack

import concourse.bass as bass
import concourse.tile as tile
from concourse import mybir, bacc
from concourse.bass_utils import run_bass_kernel_spmd
from concourse.masks import make_identity

F32 = mybir.dt.float32
F32R = mybir.dt.float32r
I32 = mybir.dt.int32
AL = mybir.AluOpType
ACTF = mybir.ActivationFunctionType

B, D, HW = 32, 256, 1024          # batch, channels, H*W
K = 2048                          # codes
NCORES = 8
BPC = B // NCORES                 # 4 batches/core
NTOK = BPC * HW                   # 4096 tokens/core
NT = NTOK // 128                  # 32 token tiles
BETA = 0.25

_CACHE = {}


def _build():
    nc = bacc.Bacc(trn_type="TRN2", target_bir_lowering=False)
    z_in = nc.declare_dram_parameter("z_in", [BPC, D, HW], F32, isOutput=False)
    cb_in = nc.declare_dram_parameter("cb_in", [K, D], F32, isOutput=False)
    zq_out = nc.declare_dram_parameter("zq_out", [NTOK, D], F32, isOutput=True)
    idx_out = nc.declare_dram_parameter("idx_out", [NTOK, 1], I32, isOutput=True)
    msum_out = nc.declare_dram_parameter("msum_out", [128, NT], F32, isOutput=True)

    with tile.TileContext(nc) as tc, ExitStack() as ctx:
        const = ctx.enter_context(tc.tile_pool(name="const", bufs=1))

        # ------- persistent tiles
        zh = [const.tile([128, NTOK], F32R, name=f"zh{c}", tag=f"zh{c}")
              for c in range(2)]
        zl = [const.tile([128, NTOK], F32R, name=f"zl{c}", tag=f"zl{c}")
              for c in range(2)]
        eh = [const.tile([128, K], F32R, name=f"eh{c}", tag=f"eh{c}")
              for c in range(2)]
        el = [const.tile([128, K], F32R, name=f"el{c}", tag=f"el{c}")
              for c in range(2)]
        bias2 = const.tile([3, NTOK], F32R, tag="bias2")    # t1h / t1l / ones
        biasmv = const.tile([3, K], F32R, tag="biasmv")     # ones / ones / t2h
        neginf = const.tile([128, 1], F32, tag="neginf")
        msum = const.tile([128, NT], F32, tag="msum")
        negk = const.tile([128, NT], F32, tag="negk")
        idxi = const.tile([128, NT], I32, tag="idxi")
        ident = const.tile([128, 128], F32, tag="ident")

        nc.vector.memset(neginf[:], -3.0e38)
        make_identity(nc, ident[:])

        # ------- prologue
        with tc.tile_pool(name="plong", bufs=1) as plong, \
             tc.tile_pool(name="ppsum", bufs=1, space="PSUM") as ppsum:

            onescol_f = plong.tile([128, 1], F32, tag="onescol_f")
            nc.vector.memset(onescol_f[:], 1.0)
            onescol = plong.tile([128, 1], F32R, tag="onescol")
            nc.vector.tensor_copy(onescol[:], onescol_f[:])
            # [128, 32] block of ones; DMA-flatten fills the ones bias rows
            onesblk_f = plong.tile([128, NT], F32, tag="onesblk_f")
            nc.vector.memset(onesblk_f[:], 1.0)
            onesblk = plong.tile([128, NT], F32R, tag="onesblk")
            nc.vector.tensor_copy(onesblk[:], onesblk_f[:])
            nc.gpsimd.dma_start(bias2[2:3, :], onesblk[:, :])
            nc.gpsimd.dma_start(biasmv[0:1, :], onesblk[0:64, :])
            nc.gpsimd.dma_start(biasmv[1:2, :], onesblk[0:64, :])

            t1col = plong.tile([128, NT], F32, tag="t1col")
            t1h = plong.tile([128, NT], F32R, tag="t1h")
            t1l = plong.tile([128, NT], F32R, tag="t1l")
            t2col = plong.tile([128, 16], F32, tag="t2col")
            t2h = plong.tile([128, 16], F32R, tag="t2h")
            sqjunk = plong.tile([128, D], F32, tag="sqjunk")
            bnc = plong.tile([NT, 128], F32R, tag="bnc")

            # ---- phase A: z slab -> zh/zl/zsq/t1
            with tc.tile_pool(name="pza", bufs=1) as pza:
                zf, zsq = [], []
                for c in range(2):
                    t = pza.tile([128, BPC, HW], F32, name=f"zf{c}", tag=f"zf{c}")
                    nc.sync.dma_start(
                        t[:],
                        z_in[:, c * 128:(c + 1) * 128, :].rearrange("b d x -> d b x"))
                    zf.append(t)
                for c in range(2):
                    zfc = zf[c][:].rearrange("p b x -> p (b x)")
                    s = pza.tile([128, NTOK], F32, name=f"zsq{c}", tag=f"zsq{c}")
                    nc.scalar.activation(out=s[:], in_=zfc, func=ACTF.Square)
                    zsq.append(s)
                    nc.vector.tensor_copy(zh[c][:], zfc)
                    nc.vector.tensor_tensor(out=zl[c][:], in0=zfc,
                                            in1=zh[c][:].bitcast(F32),
                                            op=AL.subtract)

                # t1 columns: t1col[p, t] = sum_d z[t*128+p, d]^2
                p_t1 = ppsum.tile([128, NT], F32, tag="p_t1")
                for t in range(NT):
                    sl = slice(t * 128, (t + 1) * 128)
                    nc.tensor.matmul(p_t1[:, t:t + 1], lhsT=zsq[0][:, sl],
                                     rhs=onescol_f[:], start=True, stop=False)
                    nc.tensor.matmul(p_t1[:, t:t + 1], lhsT=zsq[1][:, sl],
                                     rhs=onescol_f[:], start=False, stop=True)
                nc.vector.tensor_copy(t1col[:], p_t1[:])
                nc.vector.tensor_copy(t1h[:], t1col[:])
                nc.vector.tensor_tensor(out=t1l[:], in0=t1col[:],
                                        in1=t1h[:].bitcast(F32), op=AL.subtract)

            # ---- phase B: codebook -> eh/el/t2
            with tc.tile_pool(name="pcb", bufs=1) as pcb:
                # codebook: 16 tiles of [128 codes, 256 d] side by side
                cball = pcb.tile([128, 16 * D], F32, tag="cball")
                for i in range(16):
                    nc.sync.dma_start(cball[:, i * D:(i + 1) * D],
                                      cb_in[i * 128:(i + 1) * 128, :])

                # t2 columns: t2col[p, i] = |e_(i*128+p)|^2
                for i in range(16):
                    nc.scalar.activation(out=sqjunk[:],
                                         in_=cball[:, i * D:(i + 1) * D],
                                         func=ACTF.Square,
                                         accum_out=t2col[:, i:i + 1])
                nc.vector.tensor_copy(t2h[:], t2col[:])

                # e2n = -2 * cb^T, split hi/lo, per d-chunk
                for c in range(2):
                    p_e = ppsum.tile([128, K], F32, tag="p_e")
                    for i in range(16):
                        nc.tensor.transpose(
                            out=p_e[:, i * 128:(i + 1) * 128],
                            in_=cball[:, i * D + c * 128: i * D + (c + 1) * 128],
                            identity=ident[:])
                    e2n = pcb.tile([128, K], F32, name=f"e2n{c}", tag="e2n")
                    nc.scalar.activation(out=e2n[:], in_=p_e[:], func=ACTF.Copy,
                                         scale=-2.0)
                    nc.vector.tensor_copy(eh[c][:], e2n[:])
                    nc.vector.tensor_tensor(out=el[c][:], in0=e2n[:],
                                            in1=eh[c][:].bitcast(F32),
                                            op=AL.subtract)

            # transpose t1h/t1l/t2h columns into the bias row tiles
            for (src, n, dst) in ((t1h, NT, bias2[0:1, :]),
                                  (t1l, NT, bias2[1:2, :]),
                                  (t2h, 16, biasmv[2:3, :])):
                p_tr = ppsum.tile([NT, 128], F32, tag="p_tr")
                nc.tensor.transpose(out=p_tr[:n, :], in_=src[:, :n].bitcast(F32),
                                    identity=ident[:])
                nc.vector.tensor_copy(bnc[:n, :], p_tr[:n, :])
                nc.gpsimd.dma_start(dst, bnc[:n, :])

        # ------- main loop
        with tc.tile_pool(name="work", bufs=3) as work, \
             tc.tile_pool(name="mpsum", bufs=2, space="PSUM") as mpsum:
            for t in range(NT):
                sl = slice(t * 128, (t + 1) * 128)
                pd = mpsum.tile([128, K], F32, tag="pd")
                # stationary-major: 6 G terms then the bias term, 4 banks each
                terms = [(zh[0], eh[0]), (zh[0], el[0]), (zl[0], eh[0]),
                         (zh[1], eh[1]), (zh[1], el[1]), (zl[1], eh[1])]
                for ti, (zz, ee) in enumerate(terms):
                    for b in range(4):
                        bs = slice(b * 512, (b + 1) * 512)
                        nc.tensor.matmul(pd[:, bs], lhsT=zz[:, sl], rhs=ee[:, bs],
                                         start=(ti == 0), stop=False)
                for b in range(4):
                    bs = slice(b * 512, (b + 1) * 512)
                    nc.tensor.matmul(pd[:, bs], lhsT=bias2[:, sl],
                                     rhs=biasmv[:, bs], start=False, stop=True)

                rm = work.tile([128, K], F32, tag="rm")
                nc.vector.tensor_tensor_scan(
                    out=rm[:], data0=pd[:], data1=neginf[:].to_broadcast([128, K]),
                    initial=3.0e38, op0=AL.min, op1=AL.max)
                nc.vector.tensor_copy(msum[:, t:t + 1], rm[:, K - 1:K])

                sgn = work.tile([128, K], F32, tag="sgn")
                nc.scalar.activation(
                    out=sgn[:], in_=rm[:], func=ACTF.Sign,
                    bias=rm[:, K - 1:K], scale=-1.0,
                    accum_out=negk[:, t:t + 1])

                nc.vector.tensor_scalar(out=idxi[:, t:t + 1], in0=negk[:, t:t + 1],
                                        scalar1=-1.0, scalar2=None, op0=AL.mult)

                zqt = work.tile([128, D], F32, tag="zqt")
                nc.gpsimd.indirect_dma_start(
                    out=zqt[:], out_offset=None, in_=cb_in[:],
                    in_offset=bass.IndirectOffsetOnAxis(ap=idxi[:, t:t + 1], axis=0))
                nc.sync.dma_start(zq_out[sl, :], zqt[:])
                nc.sync.dma_start(idx_out[sl, :], idxi[:, t:t + 1])

        nc.sync.dma_start(msum_out[:], msum[:])

    nc.compile()
    return nc


def _get_nc():
    if "nc" not in _CACHE:
        _CACHE["nc"] = _build()
    return _CACHE["nc"]


def kernel(z: np.ndarray, codebook: np.ndarray):
    z = np.ascontiguousarray(z, dtype=np.float32)
    codebook = np.ascontiguousarray(codebook, dtype=np.float32)
    nc = _get_nc()

    in_maps = []
    for c in range(NCORES):
        zslab = np.ascontiguousarray(
            z[c * BPC:(c + 1) * BPC].reshape(BPC, D, HW))
        in_maps.append({"z_in": zslab, "cb_in": codebook})

    res = run_bass_kernel_spmd(nc, in_maps, core_ids=list(range(NCORES)))
    outs = res.results

    zq_flat = np.concatenate([o["zq_out"] for o in outs], axis=0)   # [32768, 256]
    idx = np.concatenate([o["idx_out"][:, 0] for o in outs], axis=0).astype(np.int32)

    total = np.float64(0.0)
    for o in outs:
        # msum[p, t] = d_min of token t*128+p on that core
        total += o["msum_out"].astype(np.float64).sum()
    e32 = np.float32(total / (B * HW * D))
    loss = np.float32(e32 + np.float32(np.float32(BETA) * e32))

    # straight-through estimator output: z + fl(z_q - z), elementwise fp32
    # (bit-exact replication of the reference's z_q_ste arithmetic)
    zq = zq_flat.reshape(B, 32, 32, D).transpose(0, 3, 1, 2)
    zq_ste = (z + (zq - z).astype(np.float32)).astype(np.float32)
    return np.ascontiguousarray(zq_ste), idx, loss
